# revision 1
# baseline (speedup 1.0000x reference)
"""Masked linear (CantorLinear): y = x @ (weight*mask).T + bias.

Structure exploited: the Cantor mask keeps ~3.9% of weights, arranged as 256
contiguous runs in the flattened (out, in) index space. Only 240 of the 2048
output rows have any nonzero weight. So the kernel packs those rows into a
[256, 2048] compact weight, computes the compact matmul on 8 NeuronCores
(data-parallel over the 16384 sequence positions), and scatters the 240
computed columns into a bias-broadcast full output on the host. The other
1808 output columns are exactly bias (filled host-side in fp32).

Device kernel (per core): y_cT[256, 2048] = W_cT.T @ x_T (+ bias per row),
with K = in_features on SBUF partitions for both operands; x is transposed
host-side so no on-device transpose is needed. Inputs stream as fp16
(x ~ N(0,1) and |W| <= 1/sqrt(2048) fit fp16's range; PSUM accumulates in
fp32), which halves HBM traffic vs fp32 at ~4.5e-4 absmax relative error.
x is additionally pre-tiled host-side ([n_si, 128, 16, NT]) so every SBUF
tile DMA moves 8KB-contiguous per partition - HW-measured 8us faster than the
512B-chunk strided layout. Measured ~38.5 us/core steady state (PE-only floor
26.6us, DMA-only 25.8us; the rest is PE<->DMA latency coupling).
"""

import os
import numpy as np

import concourse.bacc as bacc
import concourse.mybir as mybir
import concourse.tile as tile
from concourse.bass_utils import run_bass_kernel_spmd

B, SQ = 4, 4096
IN_F = 2048
OUT_F = 2048
S = B * SQ                 # 16384 flattened sequence positions
NCORES = 8
S_SH = S // NCORES         # 2048 per core
R_PAD = 256                # compact out-rows padded (240 real)
P = 128
KS = IN_F // P             # 16 k-subtiles
NT = int(os.environ.get("CANTOR_NT", "512"))   # sequence-tile width
MB = R_PAD // P            # 2 output partition blocks

# matmul input dtype: "fp16" (default), "bf16", "f32r", "f32"
MM_MODE = os.environ.get("CANTOR_MM_MODE", "fp16")
OUT_FP16 = os.environ.get("CANTOR_OUT_FP16", "1") == "1"
PRETILED = os.environ.get("CANTOR_PRETILED", "1") == "1"
# repeat the whole kernel body LOOPS times inside one NEFF (benchmarking only)
LOOPS = int(os.environ.get("CANTOR_BENCH_LOOPS", "1"))

LAST_RESULTS = None  # BassKernelResults of the most recent run (for test.py)

_NC_CACHE = {}


def _build_nc(mm_mode: str, loops: int):
    mm_cast = {
        "f32r": mybir.dt.float32r,
        "f32": mybir.dt.float32,
        "bf16": mybir.dt.bfloat16,
        "fp16": mybir.dt.float16,
    }[mm_mode]
    io_dt = mm_cast if mm_mode in ("bf16", "fp16") else mybir.dt.float32

    y_dt = mybir.dt.float16 if OUT_FP16 else mybir.dt.float32
    nc = bacc.Bacc("TRN2", target_bir_lowering=False, debug=False)
    n_si = S_SH // NT
    if PRETILED:
        xt = nc.dram_tensor("xt", [n_si, P, KS, NT], io_dt, kind="ExternalInput")
    else:
        xt = nc.dram_tensor("xt", [IN_F, S_SH], io_dt, kind="ExternalInput")
    wt = nc.dram_tensor("wt", [IN_F, R_PAD], io_dt, kind="ExternalInput")
    bc = nc.dram_tensor("bc", [R_PAD], mybir.dt.float32, kind="ExternalInput")
    yt = nc.dram_tensor("yt", [R_PAD, S_SH], y_dt, kind="ExternalOutput")

    if not PRETILED:
        xt_r = xt.rearrange("(ko p) s -> p ko s", p=P)
    wt_r = wt.rearrange("(ko p) r -> p ko r", p=P)
    bc_r = bc.rearrange("(m p) -> p m", p=P)

    with tile.TileContext(nc) as tc:
        is_f32r = mm_cast == mybir.dt.float32r
        with (
            tc.tile_pool(name="wpool", bufs=1) as wpool,
            tc.tile_pool(name="xpool", bufs=int(os.environ.get("CANTOR_XBUFS", "4"))) as xpool,
            tc.tile_pool(name="opool", bufs=int(os.environ.get("CANTOR_OBUFS", "4"))) as opool,
            tc.tile_pool(name="pspool", bufs=int(os.environ.get("CANTOR_PSBUFS", "4")), space="PSUM") as pspool,
        ):
            w_ld = wpool.tile([P, KS, R_PAD], io_dt)
            nc.sync.dma_start(w_ld[:], wt_r)
            b_sb = wpool.tile([P, MB], mybir.dt.float32)
            nc.sync.dma_start(b_sb[:], bc_r)
            if is_f32r:
                # fp32r matmul inputs must come from a rounding instruction.
                w_sb = wpool.tile([P, KS, R_PAD], mybir.dt.float32r)
                nc.vector.tensor_copy(w_sb[:], w_ld[:])
            else:
                w_sb = w_ld

            ablate = os.environ.get("CANTOR_ABLATE", "")
            evict = os.environ.get("CANTOR_EVICT", "act")
            ksplit = int(os.environ.get("CANTOR_KSPLIT", "1"))

            def body(_i=None):
                for si in range(S_SH // NT):
                    x_ld = xpool.tile([P, KS, NT], io_dt, tag="xld")
                    if ablate != "mm":
                        kh = KS // ksplit
                        for j in range(ksplit):
                            if PRETILED:
                                nc.sync.dma_start(
                                    x_ld[:, j * kh:(j + 1) * kh],
                                    xt[si, :, j * kh:(j + 1) * kh])
                            else:
                                nc.sync.dma_start(
                                    x_ld[:, j * kh:(j + 1) * kh],
                                    xt_r[:, j * kh:(j + 1) * kh,
                                         si * NT:(si + 1) * NT])
                    else:
                        nc.any.memset(x_ld[:], 0.0)
                    if is_f32r:
                        x_sb = xpool.tile([P, KS, NT], mybir.dt.float32r, tag="xr")
                        nc.vector.tensor_copy(x_sb[:], x_ld[:])
                    else:
                        x_sb = x_ld
                    for m in range(MB):
                        o_sb = opool.tile([P, NT], y_dt, tag="o")
                        if ablate == "dma":
                            nc.any.memset(o_sb[:], 0.0)
                        else:
                            ps = pspool.tile([P, NT], mybir.dt.float32, tag="ps")
                            for k in range(KS):
                                nc.tensor.matmul(
                                    ps[:],
                                    lhsT=w_sb[:, k, m * P:(m + 1) * P],
                                    rhs=x_sb[:, k, :],
                                    start=(k == 0),
                                    stop=(k == KS - 1),
                                )
                            if evict == "dve":
                                nc.vector.tensor_tensor(
                                    o_sb[:], ps[:],
                                    b_sb[:, m:m + 1].to_broadcast([P, NT]),
                                    mybir.AluOpType.add,
                                )
                            else:
                                nc.scalar.activation(
                                    o_sb[:], ps[:],
                                    mybir.ActivationFunctionType.Identity,
                                    bias=b_sb[:, m:m + 1],
                                )
                        nc.sync.dma_start(
                            yt[m * P:(m + 1) * P, si * NT:(si + 1) * NT], o_sb[:]
                        )

            if loops == 1:
                body()
            else:
                unroll = int(os.environ.get("CANTOR_BENCH_UNROLL", "1"))
                assert loops % unroll == 0
                hints = ()
                if os.environ.get("CANTOR_BENCH_HINTS", "0") == "1":
                    hints = (mybir.EngineType.PE, mybir.EngineType.SP)
                with tc.For_i(0, loops // unroll, 1, hint_engines=hints) as i:
                    for _ in range(unroll):
                        body(i)

    nc.compile()
    return nc


def _get_nc(mm_mode: str, loops: int):
    key = (mm_mode, loops)
    if key not in _NC_CACHE:
        _NC_CACHE[key] = _build_nc(mm_mode, loops)
    return _NC_CACHE[key]


def prep_in_maps(x, weight, bias, mask):
    """Host-side prep: pack compact weight/bias and per-core transposed x
    shards. Returns (in_maps, rows)."""
    x = np.asarray(x, dtype=np.float32)
    weight = np.asarray(weight, dtype=np.float32)
    bias = np.asarray(bias, dtype=np.float32)
    mask = np.asarray(mask, dtype=np.float32)

    w_eff = weight * mask
    rows = np.flatnonzero(mask.any(axis=1))
    r = len(rows)
    assert r <= R_PAD, f"compact rows {r} > padded {R_PAD}"

    if MM_MODE == "bf16":
        import ml_dtypes
        io_np = ml_dtypes.bfloat16
    elif MM_MODE == "fp16":
        io_np = np.float16
    else:
        io_np = np.float32

    w_c = np.zeros((R_PAD, IN_F), dtype=np.float32)
    w_c[:r] = w_eff[rows]
    wt = np.ascontiguousarray(w_c.T).astype(io_np)      # [IN_F, R_PAD]
    bc = np.zeros((R_PAD,), dtype=np.float32)
    bc[:r] = bias[rows]

    xf = x.reshape(S, IN_F)
    n_si = S_SH // NT
    in_maps = []
    for c in range(NCORES):
        x_t = xf[c * S_SH:(c + 1) * S_SH].T.astype(io_np)  # one-pass T + cast
        if PRETILED:
            # [IN_F, S_SH] -> [n_si, P, KS, NT]; partition-major contiguous
            x_t = np.ascontiguousarray(
                x_t.reshape(KS, P, n_si, NT).transpose(2, 1, 0, 3))
        in_maps.append({"xt": x_t, "wt": wt, "bc": bc})
    return in_maps, rows


def kernel(x, weight, bias, mask):
    global LAST_RESULTS
    bias = np.asarray(bias, dtype=np.float32)
    in_maps, rows = prep_in_maps(x, weight, bias, mask)
    r = len(rows)

    nc = _get_nc(MM_MODE, LOOPS)
    res = run_bass_kernel_spmd(nc, in_maps, list(range(NCORES)))
    LAST_RESULTS = res

    y = np.empty((S, OUT_F), dtype=np.float32)
    y[:] = bias
    for c in range(NCORES):
        y[c * S_SH:(c + 1) * S_SH, rows] = \
            res.results[c]["yt"][:r].T.astype(np.float32)
    return y.reshape(B, SQ, OUT_F)



# revision 2
# speedup vs baseline: 1.6280x; 1.6280x over previous
"""Masked linear (CantorLinear): y = x @ (weight*mask).T + bias.

Sparse flipped-orientation kernel. The Cantor mask keeps 326 contiguous
(row, k-run) pieces across 240 of the 2048 output rows. Each piece gets one
output "slot"; slots are sorted by run midpoint so that each 128-wide
k-subtile's user slots form a tight contiguous range (sum of ranges = 1529
vs 16*336 dense).

Per matmul: lhsT (stationary) = x k-subtile [128 k, 128 seq] in fp8-e3m4
(4-bit mantissa, abs err 1.2e-2 rel vs 2e-2 budget; halves x DMA vs fp16),
rhs (moving) = packed weight [128 k, range] in fp16, PSUM [128 seq, slots]
fp32 accumulates over the 16 k-subtiles. PE cost scales with the moving
free dim = slot range, so sparsity cuts PE ~2.2x vs the dense orientation.
PSUM has_written semantics (start=True clears the whole bank; start=False
overwrites where clear) make the per-subtile column offsets legal without
an init pass. Bias and the 2-piece row sums are applied host-side.

8 cores data-parallel over the 16384 sequence positions.
"""

import os
import numpy as np
import ml_dtypes

import concourse.bacc as bacc
import concourse.mybir as mybir
import concourse.tile as tile
from concourse.bass_utils import run_bass_kernel_spmd

B, SQ = 4, 4096
IN_F = 2048
OUT_F = 2048
S = B * SQ                 # 16384 flattened sequence positions
NCORES = 8
S_SH = S // NCORES         # 2048 per core
P = 128
KS = IN_F // P             # 16 k-subtiles
NT = int(os.environ.get("CANTOR_NT", "512"))   # seq positions per x DMA tile
NSI = S_SH // NT
SB = NT // P               # seq sub-blocks (psum groups) per x tile
SLOTS = 336                # 326 real slots padded

MM_MODE = os.environ.get("CANTOR_MM_MODE", "e3m4")
LOOPS = int(os.environ.get("CANTOR_BENCH_LOOPS", "1"))

LAST_RESULTS = None
_NC_CACHE = {}


def _cantor_mask(out_dim, in_dim, depth=8):
    idx = np.arange(out_dim * in_dim, dtype=np.float64)
    x = idx / (out_dim * in_dim + 1e-9)
    valid = np.ones(x.shape, dtype=bool)
    for _ in range(depth):
        x = x * 3.0
        digit = np.floor(x)
        x = x - digit
        valid &= (digit != 1.0)
    return valid.reshape(out_dim, in_dim)


def _build_tables():
    """Slot decomposition of the mask: returns (rows, slot list sorted by
    run midpoint, per-subtile [lo, hi) slot ranges, per-row slot indices)."""
    M = _cantor_mask(OUT_F, IN_F)
    rows = np.flatnonzero(M.any(axis=1))
    slots = []
    for i, r in enumerate(rows):
        m = M[r]
        d = np.diff(np.concatenate([[0], m.view(np.int8), [0]]))
        for s, e in zip(np.flatnonzero(d == 1), np.flatnonzero(d == -1)):
            slots.append((i, int(s), int(e)))
    slots.sort(key=lambda t: t[1] + t[2])
    assert len(slots) <= SLOTS
    lo = np.zeros(KS, np.int32)
    hi = np.zeros(KS, np.int32)
    for t in range(KS):
        a, b = t * P, (t + 1) * P
        idx = [j for j, (_, s, e) in enumerate(slots) if s < b and e > a]
        lo[t], hi[t] = idx[0], idx[-1] + 1
    first = np.full(len(rows), -1, np.int64)
    second = np.full(len(rows), -1, np.int64)
    for j, (i, s, e) in enumerate(slots):
        if first[i] < 0:
            first[i] = j
        else:
            second[i] = j
    return rows, slots, lo, hi, first, second


ROWS, SLOT_LIST, LO_T, HI_T, FIRST_SLOT, SECOND_SLOT = _build_tables()


def _build_nc(mm_mode, loops):
    x_dt = mybir.dt.float8e3
    nc = bacc.Bacc("TRN2", target_bir_lowering=False, debug=False)
    xt = nc.dram_tensor("xt", [NSI, P, KS, NT], x_dt, kind="ExternalInput")
    wt = nc.dram_tensor("wt", [KS, P, SLOTS], mybir.dt.float16,
                        kind="ExternalInput")
    yt = nc.dram_tensor("yt", [S_SH, SLOTS], mybir.dt.float16,
                        kind="ExternalOutput")

    with tile.TileContext(nc) as tc:
        with (
            tc.tile_pool(name="wpool", bufs=1) as wpool,
            tc.tile_pool(name="xpool",
                         bufs=int(os.environ.get("CANTOR_XBUFS", "3"))) as xpool,
            tc.tile_pool(name="opool",
                         bufs=int(os.environ.get("CANTOR_OBUFS", "4"))) as opool,
            tc.tile_pool(name="pspool",
                         bufs=int(os.environ.get("CANTOR_PSBUFS", "4")),
                         space="PSUM") as pspool,
        ):
            w_sb = wpool.tile([P, KS, SLOTS], mybir.dt.float16)
            nc.sync.dma_start(w_sb[:], wt.rearrange("t p r -> p t r"))

            ablate = os.environ.get("CANTOR_ABLATE", "")

            def body(_i=None):
                for si in range(NSI):
                    x_sb = xpool.tile([P, KS, NT], x_dt, tag="xld")
                    if ablate != "mm":
                        nc.sync.dma_start(x_sb[:], xt[si])
                    else:
                        nc.any.memset(x_sb[:], 0.0)
                    for sb in range(SB):
                        o_sb = opool.tile([P, SLOTS], mybir.dt.float16,
                                          tag="o")
                        if ablate == "dma":
                            nc.any.memset(o_sb[:], 0.0)
                        else:
                            # pad to 512 f32 = one full PSUM bank
                            ps = pspool.tile([P, 512], mybir.dt.float32,
                                             tag="ps")
                            for t in range(KS):
                                l, h = int(LO_T[t]), int(HI_T[t])
                                nc.tensor.matmul(
                                    ps[:, l:h],
                                    lhsT=x_sb[:, t, sb * P:(sb + 1) * P],
                                    rhs=w_sb[:, t, l:h],
                                    start=(t == 0),
                                    stop=(t == KS - 1),
                                    skip_group_check=True,
                                )
                            nc.scalar.activation(
                                o_sb[:], ps[:, 0:SLOTS],
                                mybir.ActivationFunctionType.Identity)
                        blk = si * SB + sb
                        nc.sync.dma_start(
                            yt[blk * P:(blk + 1) * P, :], o_sb[:])

            if loops == 1:
                body()
            else:
                unroll = int(os.environ.get("CANTOR_BENCH_UNROLL", "1"))
                assert loops % unroll == 0
                with tc.For_i(0, loops // unroll, 1) as i:
                    for _ in range(unroll):
                        body(i)

    nc.compile()
    return nc


def _get_nc(mm_mode, loops):
    key = (mm_mode, loops)
    if key not in _NC_CACHE:
        _NC_CACHE[key] = _build_nc(mm_mode, loops)
    return _NC_CACHE[key]


def _pack_weight(weight, mask):
    w_eff = (np.asarray(weight, np.float32)
             * np.asarray(mask, np.float32))[ROWS]     # [240, 2048]
    wt = np.zeros((KS, P, SLOTS), np.float16)
    for j, (i, s, e) in enumerate(SLOT_LIST):
        for t in range(s // P, (e - 1) // P + 1):
            a = max(s, t * P)
            b = min(e, (t + 1) * P)
            wt[t, a - t * P:b - t * P, j] = w_eff[i, a:b]
    return wt


def prep_in_maps(x, weight, bias, mask):
    x = np.asarray(x, dtype=np.float32)
    wt = _pack_weight(weight, mask)
    xf = x.reshape(S, IN_F)
    in_maps = []
    for c in range(NCORES):
        x_t = xf[c * S_SH:(c + 1) * S_SH].T.astype(ml_dtypes.float8_e3m4)
        # [IN_F, S_SH] -> [NSI, P, KS, NT]
        x_t = np.ascontiguousarray(
            x_t.reshape(KS, P, NSI, NT).transpose(2, 1, 0, 3))
        in_maps.append({"xt": x_t, "wt": wt})
    return in_maps, ROWS


def kernel(x, weight, bias, mask):
    global LAST_RESULTS
    bias = np.asarray(bias, dtype=np.float32)
    in_maps, rows = prep_in_maps(x, weight, bias, mask)

    nc = _get_nc(MM_MODE, LOOPS)
    res = run_bass_kernel_spmd(nc, in_maps, list(range(NCORES)))
    LAST_RESULTS = res

    sec = np.flatnonzero(SECOND_SLOT >= 0)
    y = np.empty((S, OUT_F), dtype=np.float32)
    y[:] = bias
    for c in range(NCORES):
        r = res.results[c]["yt"].astype(np.float32)   # [S_SH, SLOTS]
        acc = r[:, FIRST_SLOT]
        acc[:, sec] += r[:, SECOND_SLOT[sec]]
        y[c * S_SH:(c + 1) * S_SH, rows] = acc + bias[rows]
    return y.reshape(B, SQ, OUT_F)


# revision 5
# speedup vs baseline: 1.8850x; 1.1579x over previous
"""Masked linear (CantorLinear): y = x @ (weight*mask).T + bias.

Sparse flipped-orientation kernel. The Cantor mask keeps 326 contiguous
(row, k-run) pieces across 240 of the 2048 output rows. Each piece gets one
output "slot"; slots are sorted by run midpoint so that each 128-wide
k-subtile's user slots form a tight contiguous range (sum of ranges = 1529
vs 16*336 dense).

Per matmul: lhsT (stationary) = x k-subtile [128 k, 128 seq] in fp8-e3m4
(4-bit mantissa, abs err 1.2e-2 rel vs 2e-2 budget; halves x DMA vs fp16),
rhs (moving) = packed weight [128 k, range] in fp16, PSUM [128 seq, slots]
fp32 accumulates over the 16 k-subtiles. PE cost scales with the moving
free dim = slot range, so sparsity cuts PE ~2.2x vs the dense orientation.
PSUM has_written semantics (start=True clears the whole bank; start=False
overwrites where clear) make the per-subtile column offsets legal without
an init pass. Bias and the 2-piece row sums are applied host-side.

8 cores data-parallel over the 16384 sequence positions.
"""

import os
import numpy as np
import ml_dtypes

import concourse.bacc as bacc
import concourse.mybir as mybir
import concourse.tile as tile
from concourse.bass_utils import run_bass_kernel_spmd

B, SQ = 4, 4096
IN_F = 2048
OUT_F = 2048
S = B * SQ                 # 16384 flattened sequence positions
NCORES = 8
S_SH = S // NCORES         # 2048 per core
P = 128
KS = IN_F // P             # 16 k-subtiles
NT = int(os.environ.get("CANTOR_NT", "512"))   # seq positions per x DMA tile
NSI = S_SH // NT
SB = NT // P               # seq sub-blocks (psum groups) per x tile
SLOTS = 336                # 326 real slots padded

MM_MODE = os.environ.get("CANTOR_MM_MODE", "e3m4")
LOOPS = int(os.environ.get("CANTOR_BENCH_LOOPS", "1"))

LAST_RESULTS = None
_NC_CACHE = {}


def _cantor_mask(out_dim, in_dim, depth=8):
    idx = np.arange(out_dim * in_dim, dtype=np.float64)
    x = idx / (out_dim * in_dim + 1e-9)
    valid = np.ones(x.shape, dtype=bool)
    for _ in range(depth):
        x = x * 3.0
        digit = np.floor(x)
        x = x - digit
        valid &= (digit != 1.0)
    return valid.reshape(out_dim, in_dim)


def _build_tables():
    """Slot decomposition of the mask: returns (rows, slot list sorted by
    run midpoint, per-subtile [lo, hi) slot ranges, per-row slot indices)."""
    M = _cantor_mask(OUT_F, IN_F)
    rows = np.flatnonzero(M.any(axis=1))
    slots = []
    for i, r in enumerate(rows):
        m = M[r]
        d = np.diff(np.concatenate([[0], m.view(np.int8), [0]]))
        for s, e in zip(np.flatnonzero(d == 1), np.flatnonzero(d == -1)):
            slots.append((i, int(s), int(e)))
    slots.sort(key=lambda t: t[1] + t[2])
    assert len(slots) <= SLOTS
    lo = np.zeros(KS, np.int32)
    hi = np.zeros(KS, np.int32)
    for t in range(KS):
        a, b = t * P, (t + 1) * P
        idx = [j for j, (_, s, e) in enumerate(slots) if s < b and e > a]
        lo[t], hi[t] = idx[0], idx[-1] + 1
    first = np.full(len(rows), -1, np.int64)
    second = np.full(len(rows), -1, np.int64)
    for j, (i, s, e) in enumerate(slots):
        if first[i] < 0:
            first[i] = j
        else:
            second[i] = j
    return rows, slots, lo, hi, first, second


ROWS, SLOT_LIST, LO_T, HI_T, FIRST_SLOT, SECOND_SLOT = _build_tables()


def _build_nc(mm_mode, loops):
    x_dt = mybir.dt.float8e3
    nc = bacc.Bacc("TRN2", target_bir_lowering=False, debug=False)
    xt = nc.dram_tensor("xt", [NSI, P, KS, NT], x_dt, kind="ExternalInput")
    wt = nc.dram_tensor("wt", [KS, P, SLOTS], mybir.dt.float16,
                        kind="ExternalInput")
    # [si][sb][p][slot] — host reshapes to [S_SH, SLOTS] (same bytes)
    yt = nc.dram_tensor("yt", [NSI, SB, P, SLOTS], mybir.dt.float16,
                        kind="ExternalOutput")

    with tile.TileContext(nc) as tc:
        with (
            tc.tile_pool(name="wpool", bufs=1) as wpool,
            tc.tile_pool(name="xpool",
                         bufs=int(os.environ.get("CANTOR_XBUFS", "3"))) as xpool,
            tc.tile_pool(name="opool",
                         bufs=int(os.environ.get("CANTOR_OBUFS", "4"))) as opool,
            tc.tile_pool(name="pspool",
                         bufs=int(os.environ.get("CANTOR_PSBUFS", "4")),
                         space="PSUM") as pspool,
        ):
            w_sb = wpool.tile([P, KS, SLOTS], mybir.dt.float16)
            nc.sync.dma_start(w_sb[:], wt.rearrange("t p r -> p t r"))

            ablate = os.environ.get("CANTOR_ABLATE", "")

            def body(_i=None):
                for si in range(NSI):
                    x_sb = xpool.tile([P, KS, NT], x_dt, tag="xld")
                    if ablate != "mm":
                        nc.sync.dma_start(x_sb[:], xt[si])
                    else:
                        nc.any.memset(x_sb[:], 0.0)
                    o_sb = opool.tile([P, SB, SLOTS], mybir.dt.float16,
                                      tag="o")
                    for sb in range(SB):
                        if ablate == "dma":
                            nc.any.memset(o_sb[:, sb], 0.0)
                        else:
                            # pad to 512 f32 = one full PSUM bank
                            ps = pspool.tile([P, 512], mybir.dt.float32,
                                             tag="ps")
                            for t in range(KS):
                                l, h = int(LO_T[t]), int(HI_T[t])
                                nc.tensor.matmul(
                                    ps[:, l:h],
                                    lhsT=x_sb[:, t, sb * P:(sb + 1) * P],
                                    rhs=w_sb[:, t, l:h],
                                    start=(t == 0),
                                    stop=(t == KS - 1),
                                    skip_group_check=True,
                                )
                            nc.scalar.activation(
                                o_sb[:, sb], ps[:, 0:SLOTS],
                                mybir.ActivationFunctionType.Identity)
                    nc.sync.dma_start(
                        yt[si].rearrange("sb p r -> p sb r"), o_sb[:])

            if loops == 1:
                body()
            else:
                unroll = int(os.environ.get("CANTOR_BENCH_UNROLL", "1"))
                assert loops % unroll == 0
                with tc.For_i(0, loops // unroll, 1) as i:
                    for _ in range(unroll):
                        body(i)

    nc.compile()
    return nc


def _get_nc(mm_mode, loops):
    key = (mm_mode, loops)
    if key not in _NC_CACHE:
        _NC_CACHE[key] = _build_nc(mm_mode, loops)
    return _NC_CACHE[key]


def _pack_weight(weight, mask):
    w_eff = (np.asarray(weight, np.float32)
             * np.asarray(mask, np.float32))[ROWS]     # [240, 2048]
    wt = np.zeros((KS, P, SLOTS), np.float16)
    for j, (i, s, e) in enumerate(SLOT_LIST):
        for t in range(s // P, (e - 1) // P + 1):
            a = max(s, t * P)
            b = min(e, (t + 1) * P)
            wt[t, a - t * P:b - t * P, j] = w_eff[i, a:b]
    return wt


def prep_in_maps(x, weight, bias, mask):
    x = np.asarray(x, dtype=np.float32)
    wt = _pack_weight(weight, mask)
    xf = x.reshape(S, IN_F)
    in_maps = []
    for c in range(NCORES):
        x_t = xf[c * S_SH:(c + 1) * S_SH].T.astype(ml_dtypes.float8_e3m4)
        # [IN_F, S_SH] -> [NSI, P, KS, NT]
        x_t = np.ascontiguousarray(
            x_t.reshape(KS, P, NSI, NT).transpose(2, 1, 0, 3))
        in_maps.append({"xt": x_t, "wt": wt})
    return in_maps, ROWS


def kernel(x, weight, bias, mask):
    global LAST_RESULTS
    bias = np.asarray(bias, dtype=np.float32)
    in_maps, rows = prep_in_maps(x, weight, bias, mask)

    nc = _get_nc(MM_MODE, LOOPS)
    res = run_bass_kernel_spmd(nc, in_maps, list(range(NCORES)))
    LAST_RESULTS = res

    sec = np.flatnonzero(SECOND_SLOT >= 0)
    y = np.empty((S, OUT_F), dtype=np.float32)
    y[:] = bias
    for c in range(NCORES):
        r = res.results[c]["yt"].reshape(S_SH, SLOTS).astype(np.float32)
        acc = r[:, FIRST_SLOT]
        acc[:, sec] += r[:, SECOND_SLOT[sec]]
        y[c * S_SH:(c + 1) * S_SH, rows] = acc + bias[rows]
    return y.reshape(B, SQ, OUT_F)
